# revision 18
# baseline (speedup 1.0000x reference)
"""ChessformerAttention Trainium2 kernel (v2).

Full-input contract: kernel(**inputs) takes the unsharded inputs
(x [256,64,1024] f32, bias [1,16,64,64] f32, Wq/Wk/Wv/Wo [1024,1024] f32)
and returns the full [256,64,1024] f32 output.

Strategy: data-parallel over batch across 8 NeuronCores (32 batches each).
Host pre-work (input layout transforms only): shard x, transpose+cast to
xT bf16 [D, tokens]; cast weights to bf16; bias8 = 8*bias transposed to
[lk, h*lq] f32.

On-device pipeline per core, per super-group of 512 tokens (8 batches):
  B: qT/kT = W^T x^T ([hn, tokens]) and v = x W_v ([tokens, hn]); v is
     scattered by DMA into v_ext [lk, (h,65)] with a constant ones column
     per head (memset once at start).
  C: per batch, PSUM is preloaded with bias8, 16 score matmuls accumulate
     K^T Q on top (start=False), one exp(x/8) per 8-head group produces the
     bf16 softmax numerator, 16 output matmuls with lhsT=v_ext give the
     attention output TRANSPOSED ([hd, lq] rows 0..63) with the softmax
     denominator in row 64 (the ones column).  reciprocal of row 64 is
     DMA-broadcast across partitions, one multiply normalizes, and small
     DMAs scatter the result into outT [hn, tokens] (partition shift by
     head parity).
  E: final projection out = outT^T W_o, DMA to HBM.
Score/output matmuls use quadrant operands at partition base 64 directly
(no lo-copies); there are no PE transposes and no denominator matmuls.
Emission is software-pipelined 2 batches deep so the PE never drains.
"""

import os
import numpy as np
import ml_dtypes

KPH = os.environ.get("KPH", "BCE")  # phases to emit (bisect aid)
KC = os.environ.get("KC", "PSXON")  # C-phase steps: Preload Scores eXp Outs Norm
KSAFE = os.environ.get("KSAFE", "1") == "1"  # base-0 lo-copies for odd heads

B, L, D = 256, 64, 1024
H, HD = 16, 64
N_CORES = 8
BC = B // N_CORES            # batches per core
T = BC * L                   # tokens per core (2048)
SG = 4                       # super-groups per core
TSG = T // SG                # tokens per super-group (512)
BSG = BC // SG               # batches per super-group (8)
P = 128
KD = D // P                  # 128-row chunks of the model dim (8)
MSG = TSG // P               # 128-token chunks per super-group (4)

_compiled = None


def _build():
    import concourse.bass as bass
    import concourse.mybir as mybir
    import concourse.tile as tile
    from concourse import bacc
    from contextlib import ExitStack

    bf16 = mybir.dt.bfloat16
    f32 = mybir.dt.float32
    EXP = mybir.ActivationFunctionType.Exp

    nc = bacc.Bacc(
        "TRN2",
        target_bir_lowering=False,
        debug=False,
        enable_asserts=False,
        num_devices=N_CORES,
    )
    xt_d = nc.dram_tensor("xt", [D, T], bf16, kind="ExternalInput").ap()
    w_d = {
        name: nc.dram_tensor(name, [D, D], bf16, kind="ExternalInput").ap()
        for name in ("wq", "wk", "wv", "wo")
    }
    b8_d = nc.dram_tensor("bias8", [L, H * L], f32, kind="ExternalInput").ap()
    out_d = nc.dram_tensor("out", [T, D], f32, kind="ExternalOutput").ap()

    with tile.TileContext(nc) as tc, ExitStack() as ctx:
        wpool = ctx.enter_context(tc.tile_pool(name="w", bufs=1))
        xpool = ctx.enter_context(tc.tile_pool(name="x", bufs=1))
        cpool = ctx.enter_context(tc.tile_pool(name="c", bufs=1))
        qkpool = ctx.enter_context(tc.tile_pool(name="qk", bufs=1))
        vpool = ctx.enter_context(tc.tile_pool(name="v", bufs=1))
        opool = ctx.enter_context(tc.tile_pool(name="o", bufs=2))
        epool = ctx.enter_context(tc.tile_pool(name="e", bufs=4))
        spool = ctx.enter_context(tc.tile_pool(name="s", bufs=2))
        pp = ctx.enter_context(tc.tile_pool(name="pp", bufs=8, space="PSUM"))

        # ---- preamble DMAs, interleaved for fast PE start ----
        W = {n: [None] * KD for n in ("wq", "wk", "wv", "wo")}
        xT = [None] * KD
        for k in range(KD):
            t = wpool.tile([P, D], bf16, tag=f"wq{k}", name=f"wq{k}")
            nc.sync.dma_start(t[:], w_d["wq"][k * P:(k + 1) * P, :])
            W["wq"][k] = t
            xt = xpool.tile([P, T], bf16, tag=f"xt{k}", name=f"xt{k}")
            nc.sync.dma_start(xt[:], xt_d[k * P:(k + 1) * P, :])
            xT[k] = xt
        for name in ("wk", "wv", "wo"):
            for k in range(KD):
                t = wpool.tile([P, D], bf16, tag=f"{name}{k}", name=f"{name}{k}")
                nc.sync.dma_start(t[:], w_d[name][k * P:(k + 1) * P, :])
                W[name][k] = t
        bias8 = cpool.tile([L, H * L], f32, tag="bias8", name="bias8")
        nc.sync.dma_start(bias8[:], b8_d[:])

        # v_ext: [lk, (h, 64 hd + 1 one)] per (m, half); ones set once.
        v_ext = []
        for i in range(2 * MSG):
            ve = cpool.tile([64, H * (HD + 1)], bf16, tag=f"ve{i}", name=f"ve{i}")
            nc.any.memset(ve[:], 1.0)
            v_ext.append(ve)

        qT = [qkpool.tile([P, TSG], bf16, tag=f"qt{n}", name=f"qt{n}") for n in range(KD)]
        kT = [qkpool.tile([P, TSG], bf16, tag=f"kt{n}", name=f"kt{n}") for n in range(KD)]
        v_sb = [vpool.tile([P, D], bf16, tag=f"v{m}", name=f"v{m}") for m in range(MSG)]
        qlo = klo = None
        if KSAFE:
            qlo = [qkpool.tile([64, TSG], bf16, tag=f"ql{n}", name=f"ql{n}") for n in range(KD)]
            klo = [qkpool.tile([64, TSG], bf16, tag=f"kl{n}", name=f"kl{n}") for n in range(KD)]

        if "E" not in KPH:
            zf = spool.tile([P, 512], f32, tag="fin", name="fin", bufs=3)
            nc.any.memset(zf[:], 0.0)
            for mm_ in range(T // P):
                for n2 in range(2):
                    nc.sync.dma_start(
                        out_d[mm_ * P:(mm_ + 1) * P, n2 * 512:(n2 + 1) * 512], zf[:]
                    )

        for sg in range(SG):
            t0 = sg * TSG

            # ---- phase B: projections ----
            for wkey, dstT in (("wq", qT), ("wk", kT)):
                ps = [pp.tile([P, 512], f32, tag="mm", name="mm") for _ in range(KD)]
                for k in range(KD):
                    for n in range(KD):
                        nc.tensor.matmul(
                            ps[n][:],
                            lhsT=W[wkey][k][:, n * P:(n + 1) * P],
                            rhs=xT[k][:, t0:t0 + TSG],
                            start=(k == 0),
                            stop=(k == KD - 1),
                        )
                for n in range(KD):
                    nc.any.tensor_copy(dstT[n][:], ps[n][:])

            psv = [pp.tile([P, 512], f32, tag="mm", name="mm") for _ in range(8)]
            for k in range(KD):
                for m in range(MSG):
                    for n2 in range(2):
                        nc.tensor.matmul(
                            psv[m * 2 + n2][:],
                            lhsT=xT[k][:, t0 + m * P: t0 + (m + 1) * P],
                            rhs=W["wv"][k][:, n2 * 512:(n2 + 1) * 512],
                            start=(k == 0),
                            stop=(k == KD - 1),
                        )
            for m in range(MSG):
                for n2 in range(2):
                    nc.any.tensor_copy(v_sb[m][:, n2 * 512:(n2 + 1) * 512], psv[m * 2 + n2][:])
            for m in range(MSG):
                for half in range(2):
                    src = v_sb[m][half * 64:(half + 1) * 64, :].rearrange(
                        "p (h c) -> p h c", c=HD
                    )
                    dst = v_ext[m * 2 + half][:].rearrange(
                        "p (h c) -> p h c", c=HD + 1
                    )[:, :, 0:HD]
                    nc.sync.dma_start(dst, src)
            if KSAFE:
                for n in range(KD):
                    nc.sync.dma_start(qlo[n][:], qT[n][64:128, :])
                    nc.sync.dma_start(klo[n][:], kT[n][64:128, :])

            # ---- phase C: attention, software-pipelined 2 batches deep ----
            pscore = {}
            pout = {}
            expt = {}

            def emit_preload(b):
                pscore[b] = [pp.tile([64, 512], f32, tag="mm", name="mm") for _ in range(2)]
                nc.vector.tensor_copy(pscore[b][0][:], bias8[:, 0:512])
                nc.scalar.copy(pscore[b][1][:], bias8[:, 512:1024])

            def emit_scores(b):
                tokL = b * L
                for g in range(2):
                    for j in range(8):
                        h = g * 8 + j
                        hc, odd = h // 2, h % 2
                        if KSAFE and odd:
                            kt, qt = klo[hc], qlo[hc]
                            r = 0
                        else:
                            kt, qt = kT[hc], qT[hc]
                            r = odd * 64
                        nc.tensor.matmul(
                            pscore[b][g][:, j * 64:(j + 1) * 64],
                            lhsT=kt[r:r + 64, tokL:tokL + 64],
                            rhs=qt[r:r + 64, tokL:tokL + 64],
                            start=False,
                            stop=True,
                            skip_group_check=True,
                        )

            def emit_exp(b):
                expt[b] = []
                for g in range(2):
                    et = epool.tile([64, 512], bf16, tag="exp", name="exp")
                    nc.scalar.activation(et[:], pscore[b][g][:], EXP, scale=0.125)
                    expt[b].append(et)

            def emit_outs(b):
                pout[b] = [pp.tile([65, 512], f32, tag="mm", name="mm") for _ in range(2)]
                ve = v_ext[(b // 2) * 2 + (b % 2)]
                for g in range(2):
                    for j in range(8):
                        h = g * 8 + j
                        nc.tensor.matmul(
                            pout[b][g][:, j * 64:(j + 1) * 64],
                            lhsT=ve[:, h * (HD + 1):(h + 1) * (HD + 1)],
                            rhs=expt[b][g][:, j * 64:(j + 1) * 64],
                            start=True,
                            stop=True,
                        )

            def emit_norm(b, outT):
                tokL = b * L
                rcp = spool.tile([65, 1024], f32, tag="rcp", name="rcp")
                for g in range(2):
                    nc.vector.reciprocal(
                        rcp[64:65, g * 512:(g + 1) * 512], pout[b][g][64:65, :]
                    )
                rbc = spool.tile([64, 1024], f32, tag="rbc", name="rbc")
                nc.sync.dma_start(
                    rbc[:],
                    rcp[64:65, :][:, None, :].broadcast_to([1, 64, 1024]),
                )
                tmp = spool.tile([64, 1024], bf16, tag="tmp", name="tmp")
                for g in range(2):
                    nc.vector.tensor_mul(
                        tmp[:, g * 512:(g + 1) * 512],
                        pout[b][g][0:64, :],
                        rbc[:, g * 512:(g + 1) * 512],
                    )
                for par in range(2):
                    src = tmp[:].rearrange(
                        "p (o j two c) -> p o j two c", o=2, j=4, two=2
                    )[:, :, :, par:par + 1, :]
                    dst = outT[par * 64:(par + 1) * 64, :].rearrange(
                        "p (j t) -> p j t", t=TSG
                    )[:, :, tokL:tokL + L]
                    nc.sync.dma_start(dst, src)

            outT = opool.tile([P, KD * TSG], bf16, tag="outT", name="outT")
            if "C" in KPH:
                def pre(b):
                    if "P" in KC:
                        emit_preload(b)
                    else:
                        pscore[b] = [
                            pp.tile([64, 512], f32, tag="mm", name="mm")
                            for _ in range(2)
                        ]

                def sco(b):
                    if "S" in KC:
                        emit_scores(b)

                def ex(b):
                    if "X" in KC:
                        emit_exp(b)
                    else:
                        expt[b] = []
                        for g in range(2):
                            et = epool.tile([64, 512], bf16, tag="exp", name="exp")
                            nc.any.tensor_copy(et[:], pscore[b][g][:])
                            expt[b].append(et)

                pre(0)
                pre(1)
                sco(0)
                ex(0)
                sco(1)
                ex(1)
                for b in range(BSG):
                    if b + 2 < BSG:
                        pre(b + 2)
                    if "O" in KC:
                        emit_outs(b)
                        if "N" in KC:
                            emit_norm(b, outT)
                    if b + 2 < BSG:
                        sco(b + 2)
                        ex(b + 2)
                if "N" not in KC:
                    nc.any.memset(outT[:], 0.0)
            else:
                nc.any.memset(outT[:], 0.0)

            if "E" not in KPH:
                continue
            # ---- phase E: final projection ----
            for m in range(MSG):
                pse = [pp.tile([P, 512], f32, tag="mm", name="mm") for _ in range(2)]
                for k in range(KD):
                    for n2 in range(2):
                        nc.tensor.matmul(
                            pse[n2][:],
                            lhsT=outT[:, k * TSG + m * P: k * TSG + (m + 1) * P],
                            rhs=W["wo"][k][:, n2 * 512:(n2 + 1) * 512],
                            start=(k == 0),
                            stop=(k == KD - 1),
                        )
                for n2 in range(2):
                    fin = spool.tile([P, 512], f32, tag="fin", name="fin", bufs=3)
                    nc.any.tensor_copy(fin[:], pse[n2][:])
                    nc.sync.dma_start(
                        out_d[t0 + m * P: t0 + (m + 1) * P, n2 * 512:(n2 + 1) * 512],
                        fin[:],
                    )

    nc.compile()
    return nc


def _get_compiled():
    global _compiled
    if _compiled is None:
        _compiled = _build()
    return _compiled


def _prep_inputs(x, bias, Wq, Wk, Wv, Wo):
    bf = ml_dtypes.bfloat16
    xr = x.reshape(N_CORES, T, D)
    ws = {
        "wq": np.ascontiguousarray(Wq.astype(bf)),
        "wk": np.ascontiguousarray(Wk.astype(bf)),
        "wv": np.ascontiguousarray(Wv.astype(bf)),
        "wo": np.ascontiguousarray(Wo.astype(bf)),
    }
    b8 = (8.0 * bias[0].astype(np.float32)).transpose(2, 0, 1).reshape(L, H * L)
    b8 = np.ascontiguousarray(b8)
    in_maps = [
        {"xt": np.ascontiguousarray(xr[c].T.astype(bf)), "bias8": b8, **ws}
        for c in range(N_CORES)
    ]
    return in_maps


def kernel(x, bias, Wq, Wk, Wv, Wo, _trace=False, _trace_kwargs=None):
    from concourse.bass_utils import run_bass_kernel_spmd

    nc = _get_compiled()
    in_maps = _prep_inputs(
        np.asarray(x, dtype=np.float32),
        np.asarray(bias, dtype=np.float32),
        np.asarray(Wq, dtype=np.float32),
        np.asarray(Wk, dtype=np.float32),
        np.asarray(Wv, dtype=np.float32),
        np.asarray(Wo, dtype=np.float32),
    )
    res = run_bass_kernel_spmd(
        nc, in_maps, list(range(N_CORES)), trace=_trace, **(_trace_kwargs or {})
    )
    out = np.stack([np.asarray(res.results[c]["out"]) for c in range(N_CORES)])
    out = out.reshape(B, L, D).astype(np.float32)
    if _trace:
        return out, res
    return out
